# revision 12
# baseline (speedup 1.0000x reference)
"""Bidirectional-LSTM Trainium2 kernel (nn_BLSTM).

Problem: B=64,T=512,D=H=512. Two independent LSTMs (forward input x_f,
backward input x_b, both scanned t=0..T-1), outputs summed, then two
H x H linear layers (no nonlinearity between them -> collapsed into one
matmul with W21 = W2 @ W1, b21 = W2 @ b1 + b2).

Sharding (8 cores, fully SPMD - same program, different data):
  core r: direction = r % 2 (0 -> f, 1 -> b), batch shard = r // 2
  each core runs one LSTM direction for 16 batches, then applies the
  fused linear; the host sums the per-direction partial outputs.

On-core layout: hidden/gate dim on partitions, batch on the free dim.
Per step the gate pre-activations live in TWO psum tiles in separate
banks (gps_A / gps_B, one per hidden-half hc 0,1 | hc 2,3), each
[128,128] with columns [i|f|o|g] x (2 hc x 16 lanes). The input
projection xg_t (bias folded in, precomputed chunk-ahead into SBUF
bf16) is preloaded into each psum tile by an identity-weight matmul
(start=True); the 64 recurrence matmuls accumulate on top
(start=False) so no DVE add sits on the critical path. Matmuls are
ordered (k0,k1 | k2,k3) x j-half: each consumes only the h-half
already written, the A-half gates complete after 32 matmuls so the
A-half elementwise chain overlaps the B-half matmul stream, and the
B-half chain overlaps the next step's A matmuls. Keeping the PE
stream dense also holds the HAM clock at 2.4 GHz and lets the 64
per-step LDWEIGHTS (one per 128x128 Whh tile) pipeline behind the
matmuls instead of serializing after the h dependency.

Elementwise per half (critical path = instruction count x ~200-300ns
of sequencer/semaphore overhead, so ops are fused to the minimum):
one ACT sigmoid [128,128] over all four gate blocks (g rows
pre-scaled by 2 so tanh(g) = 2*sig(2g)-1), then a 4-op custom-DVE
chain with cheap dispatch -- t1 = (2*sig(2g)-1)*sig(i) and
cm = sig(f)*c via affine_mul_reduce, c = t1 + cm via affine_then_add
-- then ACT sig(2c) and h = (2*sig(2c)-1)*sig(o) via one more
affine_mul_reduce into the bf16 ring. Each accumulator tile gets its
own pool tag: sharing one tag creates WAW false deps that serialize
the chain. Proj/linear matmuls (N=512) are interleaved as PE filler
between steps.

SBUF pools run deeper than strictly needed (acts 6, small 8, c 4,
ring 3, xg/xch 3, evac 4 bufs) purely to relax WAR edges for the
scheduler.

Measured on trn2 (8 cores, in-band repeat differential vs R=3
program, drift-cancelled): ~1.33-1.63 ms vs 7.733 ms for the
original baseline (~5x).
"""

import functools
import numpy as np
import ml_dtypes

import concourse.bass as bass
import concourse.tile as tile
from concourse import bacc, mybir
from concourse.bass_utils import run_bass_kernel_spmd

# ---------------- problem constants ----------------
B, T, D, H = 64, 512, 512, 512
G = 4 * H                 # 2048 gate dim
N_CORES = 8
BL = B // (N_CORES // 2)  # 16 local batch per core
TC = 32                   # timesteps per chunk
NCH = T // TC             # chunks
# torch gate idx (i,f,g,o) -> 32-col block within a half [i|f|o|g]
BLK2 = {0: 0, 1: 1, 2: 3, 3: 2}

WEIGHT_DTYPE = "bfloat16"

F32 = mybir.dt.float32
AFT = mybir.ActivationFunctionType
ALU = mybir.AluOpType
MULT = mybir.AluOpType.mult


def _dt():
    return F32 if WEIGHT_DTYPE == "float32" else mybir.dt.bfloat16


def _np_wdt():
    return np.float32 if WEIGHT_DTYPE == "float32" else ml_dtypes.bfloat16


def _build_program(chunks=None, repeats=1):
    if chunks is None:
        chunks = NCH
    wdt = _dt()
    nc = bacc.Bacc("TRN2", target_bir_lowering=False, debug=False,
                   num_devices=N_CORES)

    xT_d = nc.dram_tensor("xT", [4, 128, T, BL], wdt, kind="ExternalInput").ap()
    wih_d = nc.dram_tensor("wih", [4, 128, G], wdt, kind="ExternalInput").ap()
    whh_d = nc.dram_tensor("whh", [4, 128, G], wdt, kind="ExternalInput").ap()
    w21_d = nc.dram_tensor("w21", [4, 128, H], wdt, kind="ExternalInput").ap()
    biasg_d = nc.dram_tensor("biasg", [128, 16], F32, kind="ExternalInput").ap()
    brow_d = nc.dram_tensor("brow", [1, G], wdt, kind="ExternalInput").ap()
    ones_d = nc.dram_tensor("ones", [1, TC * BL], wdt, kind="ExternalInput").ap()
    b21_d = nc.dram_tensor("b21", [128, 4], F32, kind="ExternalInput").ap()
    h0_d = nc.dram_tensor("h0p", [128, 64], wdt, kind="ExternalInput").ap()
    c0_d = nc.dram_tensor("c0p", [128, 64], F32, kind="ExternalInput").ap()
    iden_d = nc.dram_tensor("iden", [128, 128], wdt, kind="ExternalInput").ap()
    pred_d = nc.dram_tensor("predT", [H, T * BL], F32, kind="ExternalOutput").ap()

    with tile.TileContext(nc) as tc:
        with (
            tc.tile_pool(name="const", bufs=1) as cpool,
            tc.tile_pool(name="xch", bufs=3) as xch_pool,
            tc.tile_pool(name="xg", bufs=3) as xg_pool,
            tc.tile_pool(name="ring", bufs=3) as ring_pool,
            tc.tile_pool(name="acts", bufs=6) as acts_pool,
            tc.tile_pool(name="small", bufs=8) as small_pool,
            tc.tile_pool(name="cstate", bufs=4) as c_pool,
            tc.tile_pool(name="evac", bufs=4) as evac_pool,
            tc.tile_pool(name="gpsA", bufs=2, space="PSUM") as gpsA_pool,
            tc.tile_pool(name="gpsB", bufs=2, space="PSUM") as gpsB_pool,
            tc.tile_pool(name="pps", bufs=2, space="PSUM") as pps_pool,
            tc.tile_pool(name="lps", bufs=2, space="PSUM") as lps_pool,
        ):
            # ---- preload constants ----
            whh_sb = cpool.tile([128, 4 * G], wdt, tag="whh")
            wih_sb = cpool.tile([128, 4 * G], wdt, tag="wih")
            w21_sb = cpool.tile([128, 4 * H], wdt, tag="w21")
            biasg_sb = cpool.tile([128, 16], F32, tag="biasg")
            b21_sb = cpool.tile([128, 4], F32, tag="b21")
            h0_sb = cpool.tile([128, 64], wdt, tag="h0")
            c0_sb = cpool.tile([128, 64], F32, tag="c0")
            iden_sb = cpool.tile([128, 128], wdt, tag="iden")
            brow_sb = cpool.tile([1, G], wdt, tag="brow")
            ones_sb = cpool.tile([1, TC * BL], wdt, tag="ones")
            for kc in range(4):
                nc.gpsimd.dma_start(whh_sb[:, kc * G:(kc + 1) * G], whh_d[kc])
                nc.gpsimd.dma_start(wih_sb[:, kc * G:(kc + 1) * G], wih_d[kc])
                nc.gpsimd.dma_start(w21_sb[:, kc * H:(kc + 1) * H], w21_d[kc])
            nc.gpsimd.dma_start(biasg_sb[:], biasg_d[:])
            nc.gpsimd.dma_start(b21_sb[:], b21_d[:])
            nc.gpsimd.dma_start(h0_sb[:], h0_d[:])
            nc.gpsimd.dma_start(c0_sb[:], c0_d[:])
            nc.gpsimd.dma_start(iden_sb[:], iden_d[:])
            nc.gpsimd.dma_start(brow_sb[:], brow_d[:])
            nc.gpsimd.dma_start(ones_sb[:], ones_d[:])

            # ---- projection helpers ----
            def proj_dma(ch):
                xch = xch_pool.tile([128, 4 * TC * BL], wdt, tag="xch")
                for dc in range(4):
                    nc.gpsimd.dma_start(
                        xch[:, dc * TC * BL:(dc + 1) * TC * BL],
                        xT_d[dc, :, ch * TC:(ch + 1) * TC, :])
                return xch

            def xg_off(jc):
                g_idx, hc = jc // 4, jc % 4
                return (hc // 2) * 128 + BLK2[g_idx] * 32 + (hc % 2) * 16

            def proj_bias_mm(pp, jc):
                # rank-1: gates_j += bias_j (x) ones  (k = 1 partition)
                nc.tensor.matmul(
                    pp[:], brow_sb[:, jc * 128:(jc + 1) * 128], ones_sb[:],
                    start=True, stop=False, skip_group_check=True)

            def proj_mm(pp, xch, jc, dc):
                nc.tensor.matmul(
                    pp[:],
                    wih_sb[:, dc * G + jc * 128: dc * G + (jc + 1) * 128],
                    xch[:, dc * TC * BL:(dc + 1) * TC * BL],
                    start=False, stop=(dc == 3), skip_group_check=True)

            def proj_evac(pp, xg, jc):
                off = xg_off(jc)
                dst = xg[:].rearrange("p (t c) -> p t c", c=256)[:, :, off:off + 16]
                nc.vector.tensor_copy(dst, pp[:])

            def linear_group(ring_src, ch_src, jc):
                lp = lps_pool.tile([128, TC * BL], F32, tag="lp")
                r3 = ring_src[:].rearrange("p (t c) -> p t c", c=64)
                for kc in range(4):
                    nc.tensor.matmul(
                        lp[:],
                        w21_sb[:, kc * H + jc * 128: kc * H + (jc + 1) * 128],
                        r3[:, :, kc * 16:(kc + 1) * 16],
                        start=(kc == 0), stop=(kc == 3))
                linear_evac(lp, ch_src, jc)

            def linear_evac(lp, ch_src, jc):
                ev = evac_pool.tile([128, TC * BL], F32, tag="ev")
                nc.scalar.activation(ev[:], lp[:], AFT.Identity,
                                     bias=b21_sb[:, jc:jc + 1])
                nc.gpsimd.dma_start(
                    pred_d[jc * 128:(jc + 1) * 128,
                           ch_src * TC * BL:(ch_src + 1) * TC * BL], ev[:])

            # per-half elementwise: gates psum -> h/2 half in ring (bf16).
            # Cheap-dispatch ops only; cm runs on GpSimd in parallel with
            # t1h on Vector.  h is stored as h/2 (host pre-scales Whh/W21
            # by 2 and halves h0), so every mul fits the native
            # (in0 - 0.5) * in1 tensor-scalar-tensor form.
            def elem_half(gps, c_prev, c_new, ring, tl, hf):
                acts = acts_pool.tile([128, 128], F32, tag="acts")
                nc.scalar.activation(acts[:], gps[:, 0:128], AFT.Sigmoid)
                i_, f_ = acts[:, 0:32], acts[:, 32:64]
                o_, g_ = acts[:, 64:96], acts[:, 96:128]
                cs = slice(hf * 32, (hf + 1) * 32)
                t1h = small_pool.tile([128, 32], F32, tag="t1h_%d" % hf)
                # t1h = (sig(2g) - 0.5) * sig(i)  (= tanh(g)*sig(i)/2)
                nc.vector.scalar_tensor_tensor(t1h[:], g_, 0.5, i_,
                                               ALU.subtract, ALU.mult)
                cm = small_pool.tile([128, 32], F32, tag="cm_%d" % hf)
                nc.gpsimd.tensor_tensor(cm[:], f_, c_prev[:, cs], ALU.mult)
                # c_new = 2*t1h + cm
                nc.vector.affine_then_add(c_new[:, cs], t1h[:], cm[:], 2.0, 0.0)
                s2c = small_pool.tile([128, 32], F32, tag="s2c_%d" % hf)
                nc.scalar.activation(s2c[:], c_new[:, cs], AFT.Sigmoid, scale=2.0)
                # h/2 = (s2c - 0.5) * sig(o) -> bf16
                nc.vector.scalar_tensor_tensor(
                    ring[:, tl * 64 + hf * 32: tl * 64 + (hf + 1) * 32],
                    s2c[:], 0.5, o_, ALU.subtract, ALU.mult)

            # j-groups per half: half 0 -> hc 0,1 ; half 1 -> hc 2,3
            HALF_JC = {0: [g * 4 + hc for g in range(4) for hc in (0, 1)],
                       1: [g * 4 + hc for g in range(4) for hc in (2, 3)]}

            def run_once():
                # ---- prologue: project chunk 0 ----
                xch = proj_dma(0)
                xg_cur = xg_pool.tile([128, TC * 256], wdt, tag="xg")
                for jc in range(16):
                    pp = pps_pool.tile([128, TC * BL], F32, tag="pp")
                    proj_bias_mm(pp, jc)
                    for dc in range(4):
                        proj_mm(pp, xch, jc, dc)
                    proj_evac(pp, xg_cur, jc)

                c_prev = c0_sb
                prev_ring = None
                xg_next = None
                pp_cur = None
                for ch in range(chunks):
                    ring = ring_pool.tile([128, TC * 64], wdt, tag="ring")
                    for tl in range(TC):
                        if tl > 0:
                            hsrc, hoff = ring, (tl - 1) * 64
                        elif ch > 0:
                            hsrc, hoff = prev_ring, (TC - 1) * 64
                        else:
                            hsrc, hoff = h0_sb, 0
                        have_proj = ch + 1 < chunks
                        if have_proj and tl == 0:
                            xch = proj_dma(ch + 1)
                            xg_next = xg_pool.tile([128, TC * 256], wdt, tag="xg")

                        # ---- psum preload: xg_t via identity matmul ----
                        # full-bank tiles: a start=True matmul clears its
                        # whole PSUM bank, so two rotation bufs sharing one
                        # bank would serialize preload(t+1) behind the
                        # sigmoid reads of step t.  Pad to 2KB/partition.
                        gpsA = gpsA_pool.tile([128, 512], F32, tag="ga")
                        gpsB = gpsB_pool.tile([128, 512], F32, tag="gb")
                        xbase = tl * 256
                        nc.tensor.matmul(gpsA[:, 0:128], iden_sb[:],
                                         xg_cur[:, xbase:xbase + 128],
                                         start=True, stop=False)
                        nc.tensor.matmul(gpsB[:, 0:128], iden_sb[:],
                                         xg_cur[:, xbase + 128:xbase + 256],
                                         start=True, stop=False)

                        # ---- proj filler piece 1 (next chunk) ----
                        if have_proj:
                            pidx = tl * 2
                            jc_f, dc_f = pidx // 4, pidx % 4
                            if dc_f == 0:
                                pp_cur = pps_pool.tile([128, TC * BL], F32, tag="pp")
                                proj_bias_mm(pp_cur, jc_f)
                            proj_mm(pp_cur, xch, jc_f, dc_f)

                        # ---- recurrence matmuls, (k01 | k23) x j-half ----
                        for hf in (0, 1):
                            gps = gpsA if hf == 0 else gpsB
                            jcs = HALF_JC[hf]
                            for kpair in ((0, 1), (2, 3)):
                                for kc in kpair:
                                    for j_i, jc in enumerate(jcs):
                                        g_idx, hc = jc // 4, jc % 4
                                        off = BLK2[g_idx] * 32 + (hc % 2) * 16
                                        last = (kc == 3 and j_i == len(jcs) - 1)
                                        nc.tensor.matmul(
                                            gps[:, off:off + 16],
                                            whh_sb[:, kc * G + jc * 128:
                                                   kc * G + (jc + 1) * 128],
                                            hsrc[:, hoff + kc * 16:
                                                 hoff + (kc + 1) * 16],
                                            start=False, stop=last,
                                            skip_group_check=True)

                        # ---- proj filler piece 2 ----
                        if have_proj:
                            pidx = tl * 2 + 1
                            jc_f, dc_f = pidx // 4, pidx % 4
                            proj_mm(pp_cur, xch, jc_f, dc_f)
                            if dc_f == 3:
                                proj_evac(pp_cur, xg_next, jc_f)

                        # ---- linear filler (previous chunk), 1 mm/step ----
                        if ch >= 1 and 3 <= tl < 19:
                            li = tl - 3
                            jc_l, kc_l = li // 4, li % 4
                            if kc_l == 0:
                                lp_cur = lps_pool.tile([128, TC * BL], F32, tag="lp")
                            r3 = prev_ring[:].rearrange("p (t c) -> p t c", c=64)
                            nc.tensor.matmul(
                                lp_cur[:],
                                w21_sb[:, kc_l * H + jc_l * 128:
                                       kc_l * H + (jc_l + 1) * 128],
                                r3[:, :, kc_l * 16:(kc_l + 1) * 16],
                                start=(kc_l == 0), stop=(kc_l == 3))
                            if kc_l == 3:
                                linear_evac(lp_cur, ch - 1, jc_l)

                        # ---- gate nonlinearities + state update (halves) ----
                        c_new = c_pool.tile([128, 64], F32, tag="c")
                        elem_half(gpsA, c_prev, c_new, ring, tl, 0)
                        elem_half(gpsB, c_prev, c_new, ring, tl, 1)
                        c_prev = c_new
                    prev_ring = ring
                    if ch + 1 < chunks:
                        xg_cur = xg_next
                # epilogue: linear for the last chunk
                for jc in range(4):
                    linear_group(prev_ring, chunks - 1, jc)


            for _rep in range(repeats):
                run_once()

    nc.compile()
    return nc


@functools.lru_cache(maxsize=4)
def _get_program(chunks=None, repeats=1):
    return _build_program(chunks, repeats)


def _pack_core_inputs(x, h0, c0, Wih, Whh, bias, W21, b21_or_zero):
    """Host-side layout prep for one core. x:[BL,T,D], h0/c0:[BL,H]."""
    npw = _np_wdt()
    # g-gate rows [2H,3H) pre-scaled by 2: tanh(g) computed as 2*sig(2g)-1
    Wih = np.concatenate([Wih[:2 * H], Wih[2 * H:3 * H] * 2.0, Wih[3 * H:]])
    Whh = np.concatenate([Whh[:2 * H], Whh[2 * H:3 * H] * 2.0, Whh[3 * H:]])
    bias = np.concatenate([bias[:2 * H], bias[2 * H:3 * H] * 2.0, bias[3 * H:]])
    # h is stored as h/2 in-kernel: scale the h-consuming weights by 2
    Whh = Whh * 2.0
    W21 = W21 * 2.0
    h0 = h0 * 0.5
    xT = np.ascontiguousarray(
        x.transpose(2, 1, 0).reshape(4, 128, T, BL)).astype(npw)
    wih = np.ascontiguousarray(Wih.T.reshape(4, 128, G)).astype(npw)
    whh = np.ascontiguousarray(Whh.T.reshape(4, 128, G)).astype(npw)
    w21 = np.ascontiguousarray(W21.T.reshape(4, 128, H)).astype(npw)
    biasg = np.ascontiguousarray(bias.reshape(16, 128).T).astype(np.float32)
    brow = np.ascontiguousarray(bias.reshape(1, G)).astype(npw)
    ones = np.ones((1, TC * BL), npw)
    b21v = np.ascontiguousarray(b21_or_zero.reshape(4, 128).T).astype(np.float32)
    h0p = np.ascontiguousarray(
        h0.T.reshape(4, 128, BL).transpose(1, 0, 2).reshape(128, 64)).astype(npw)
    c0p = np.ascontiguousarray(
        c0.T.reshape(4, 128, BL).transpose(1, 0, 2).reshape(128, 64)).astype(np.float32)
    iden = np.eye(128, dtype=npw)
    return {"xT": xT, "wih": wih, "whh": whh, "w21": w21, "biasg": biasg,
            "brow": brow, "ones": ones,
            "b21": b21v, "h0p": h0p, "c0p": c0p, "iden": iden}


def _make_in_maps(inputs):
    f32 = np.float32
    x_f = np.asarray(inputs["x_f"], f32)
    x_b = np.asarray(inputs["x_b"], f32)
    h0_f, c0_f = np.asarray(inputs["h0_f"], f32), np.asarray(inputs["c0_f"], f32)
    h0_b, c0_b = np.asarray(inputs["h0_b"], f32), np.asarray(inputs["c0_b"], f32)
    Wih_f, Whh_f = np.asarray(inputs["Wih_f"], f32), np.asarray(inputs["Whh_f"], f32)
    Wih_b, Whh_b = np.asarray(inputs["Wih_b"], f32), np.asarray(inputs["Whh_b"], f32)
    bias_f = np.asarray(inputs["bih_f"], f32) + np.asarray(inputs["bhh_f"], f32)
    bias_b = np.asarray(inputs["bih_b"], f32) + np.asarray(inputs["bhh_b"], f32)
    W1, b1 = np.asarray(inputs["W1"], f32), np.asarray(inputs["b1"], f32)
    W2, b2 = np.asarray(inputs["W2"], f32), np.asarray(inputs["b2"], f32)

    W21 = (W2 @ W1).astype(f32)          # pred = out @ W21.T + b21
    b21 = (W2 @ b1 + b2).astype(f32)
    zeros = np.zeros_like(b21)

    in_maps = []
    for r in range(N_CORES):
        d, s = r % 2, r // 2
        sl = slice(s * BL, (s + 1) * BL)
        if d == 0:
            in_maps.append(_pack_core_inputs(
                x_f[sl], h0_f[sl], c0_f[sl], Wih_f, Whh_f, bias_f, W21, b21))
        else:
            in_maps.append(_pack_core_inputs(
                x_b[sl], h0_b[sl], c0_b[sl], Wih_b, Whh_b, bias_b, W21, zeros))
    return in_maps


def _assemble(results):
    out = np.empty((B, T, H), np.float32)
    for s in range(N_CORES // 2):
        sT = results[2 * s]["predT"] + results[2 * s + 1]["predT"]
        out[s * BL:(s + 1) * BL] = sT.reshape(H, T, BL).transpose(2, 1, 0)
    return out.reshape(B * T, H)


def kernel(x_f, x_b, h0_f, c0_f, h0_b, c0_b,
           Wih_f, Whh_f, bih_f, bhh_f,
           Wih_b, Whh_b, bih_b, bhh_b,
           W1, b1, W2, b2):
    in_maps = _make_in_maps(dict(
        x_f=x_f, x_b=x_b, h0_f=h0_f, c0_f=c0_f, h0_b=h0_b, c0_b=c0_b,
        Wih_f=Wih_f, Whh_f=Whh_f, bih_f=bih_f, bhh_f=bhh_f,
        Wih_b=Wih_b, Whh_b=Whh_b, bih_b=bih_b, bhh_b=bhh_b,
        W1=W1, b1=b1, W2=W2, b2=b2))
    nc = _get_program()
    res = run_bass_kernel_spmd(nc, in_maps, core_ids=list(range(N_CORES)))
    return _assemble(res.results)



# revision 13
# speedup vs baseline: 1.0086x; 1.0086x over previous
"""Bidirectional-LSTM Trainium2 kernel (nn_BLSTM).

Problem: B=64,T=512,D=H=512. Two independent LSTMs (forward input x_f,
backward input x_b, both scanned t=0..T-1), outputs summed, then two
H x H linear layers (no nonlinearity between them -> collapsed into one
matmul with W21 = W2 @ W1, b21 = W2 @ b1 + b2).

Sharding (8 cores, fully SPMD - same program, different data):
  core r: direction = r % 2 (0 -> f, 1 -> b), batch shard = r // 2
  each core runs one LSTM direction for 16 batches, then applies the
  fused linear; the host sums the per-direction partial outputs.

On-core layout: hidden/gate dim on partitions, batch on the free dim.
Per step the gate pre-activations live in TWO psum tiles in separate
banks (gps_A / gps_B, one per hidden-half hc 0,1 | hc 2,3), each
[128,128] with columns [i|f|o|g] x (2 hc x 16 lanes). The input
projection xg_t (bias folded in, precomputed chunk-ahead into SBUF
bf16) is preloaded into each psum tile by an identity-weight matmul
(start=True); the 64 recurrence matmuls accumulate on top
(start=False) so no DVE add sits on the critical path. Matmuls are
ordered (k0,k1 | k2,k3) x j-half: each consumes only the h-half
already written, the A-half gates complete after 32 matmuls so the
A-half elementwise chain overlaps the B-half matmul stream, and the
B-half chain overlaps the next step's A matmuls. Keeping the PE
stream dense also holds the HAM clock at 2.4 GHz and lets the 64
per-step LDWEIGHTS (one per 128x128 Whh tile) pipeline behind the
matmuls instead of serializing after the h dependency.

Elementwise per half (critical path = instruction count x ~200-300ns
of sequencer/semaphore overhead, so ops are fused to the minimum):
one ACT sigmoid [128,128] over all four gate blocks (g rows
pre-scaled by 2 so tanh(g) = 2*sig(2g)-1), then a 4-op custom-DVE
chain with cheap dispatch -- t1 = (2*sig(2g)-1)*sig(i) and
cm = sig(f)*c via affine_mul_reduce, c = t1 + cm via affine_then_add
-- then ACT sig(2c) and h = (2*sig(2c)-1)*sig(o) via one more
affine_mul_reduce into the bf16 ring. Each accumulator tile gets its
own pool tag: sharing one tag creates WAW false deps that serialize
the chain. Proj/linear matmuls (N=512) are interleaved as PE filler
between steps.

SBUF pools run deeper than strictly needed (acts 6, small 8, c 4,
ring 3, xg/xch 3, evac 4 bufs) purely to relax WAR edges for the
scheduler.

Measured on trn2 (8 cores, in-band repeat differential vs R=3
program, drift-cancelled): ~1.33-1.63 ms vs 7.733 ms for the
original baseline (~5x).
"""

import functools
import numpy as np
import ml_dtypes

import concourse.bass as bass
import concourse.tile as tile
from concourse import bacc, mybir
from concourse.bass_utils import run_bass_kernel_spmd

# ---------------- problem constants ----------------
B, T, D, H = 64, 512, 512, 512
G = 4 * H                 # 2048 gate dim
N_CORES = 8
BL = B // (N_CORES // 2)  # 16 local batch per core
TC = 32                   # timesteps per chunk
NCH = T // TC             # chunks
# torch gate idx (i,f,g,o) -> 32-col block within a half [i|f|o|g]
BLK2 = {0: 0, 1: 1, 2: 3, 3: 2}

WEIGHT_DTYPE = "bfloat16"

F32 = mybir.dt.float32
AFT = mybir.ActivationFunctionType
ALU = mybir.AluOpType
MULT = mybir.AluOpType.mult


def _dt():
    return F32 if WEIGHT_DTYPE == "float32" else mybir.dt.bfloat16


def _np_wdt():
    return np.float32 if WEIGHT_DTYPE == "float32" else ml_dtypes.bfloat16


def _build_program(chunks=None, repeats=1):
    if chunks is None:
        chunks = NCH
    wdt = _dt()
    nc = bacc.Bacc("TRN2", target_bir_lowering=False, debug=False,
                   num_devices=N_CORES)

    xT_d = nc.dram_tensor("xT", [4, 128, T, BL], wdt, kind="ExternalInput").ap()
    wih_d = nc.dram_tensor("wih", [4, 128, G], wdt, kind="ExternalInput").ap()
    whh_d = nc.dram_tensor("whh", [4, 128, G], mybir.dt.float8e4, kind="ExternalInput").ap()
    w21_d = nc.dram_tensor("w21", [4, 128, H], wdt, kind="ExternalInput").ap()
    biasg_d = nc.dram_tensor("biasg", [128, 16], F32, kind="ExternalInput").ap()
    brow_d = nc.dram_tensor("brow", [1, G], wdt, kind="ExternalInput").ap()
    ones_d = nc.dram_tensor("ones", [1, TC * BL], wdt, kind="ExternalInput").ap()
    b21_d = nc.dram_tensor("b21", [128, 4], F32, kind="ExternalInput").ap()
    h0_d = nc.dram_tensor("h0p", [128, 64], wdt, kind="ExternalInput").ap()
    c0_d = nc.dram_tensor("c0p", [128, 64], F32, kind="ExternalInput").ap()
    iden_d = nc.dram_tensor("iden", [128, 128], wdt, kind="ExternalInput").ap()
    pred_d = nc.dram_tensor("predT", [H, T * BL], F32, kind="ExternalOutput").ap()

    with tile.TileContext(nc) as tc:
        with (
            tc.tile_pool(name="const", bufs=1) as cpool,
            tc.tile_pool(name="xch", bufs=3) as xch_pool,
            tc.tile_pool(name="xg", bufs=3) as xg_pool,
            tc.tile_pool(name="ring", bufs=3) as ring_pool,
            tc.tile_pool(name="acts", bufs=6) as acts_pool,
            tc.tile_pool(name="small", bufs=8) as small_pool,
            tc.tile_pool(name="cstate", bufs=4) as c_pool,
            tc.tile_pool(name="evac", bufs=4) as evac_pool,
            tc.tile_pool(name="gpsA", bufs=2, space="PSUM") as gpsA_pool,
            tc.tile_pool(name="gpsB", bufs=2, space="PSUM") as gpsB_pool,
            tc.tile_pool(name="pps", bufs=2, space="PSUM") as pps_pool,
            tc.tile_pool(name="lps", bufs=2, space="PSUM") as lps_pool,
        ):
            # ---- preload constants ----
            whh_sb = cpool.tile([128, 4 * G], mybir.dt.float8e4, tag="whh")
            wih_sb = cpool.tile([128, 4 * G], wdt, tag="wih")
            w21_sb = cpool.tile([128, 4 * H], wdt, tag="w21")
            biasg_sb = cpool.tile([128, 16], F32, tag="biasg")
            b21_sb = cpool.tile([128, 4], F32, tag="b21")
            h0_sb = cpool.tile([128, 64], wdt, tag="h0")
            c0_sb = cpool.tile([128, 64], F32, tag="c0")
            iden_sb = cpool.tile([128, 128], wdt, tag="iden")
            brow_sb = cpool.tile([1, G], wdt, tag="brow")
            ones_sb = cpool.tile([1, TC * BL], wdt, tag="ones")
            for kc in range(4):
                nc.gpsimd.dma_start(whh_sb[:, kc * G:(kc + 1) * G], whh_d[kc])
                nc.gpsimd.dma_start(wih_sb[:, kc * G:(kc + 1) * G], wih_d[kc])
                nc.gpsimd.dma_start(w21_sb[:, kc * H:(kc + 1) * H], w21_d[kc])
            nc.gpsimd.dma_start(biasg_sb[:], biasg_d[:])
            nc.gpsimd.dma_start(b21_sb[:], b21_d[:])
            nc.gpsimd.dma_start(h0_sb[:], h0_d[:])
            nc.gpsimd.dma_start(c0_sb[:], c0_d[:])
            nc.gpsimd.dma_start(iden_sb[:], iden_d[:])
            nc.gpsimd.dma_start(brow_sb[:], brow_d[:])
            nc.gpsimd.dma_start(ones_sb[:], ones_d[:])

            # ---- projection helpers ----
            def proj_dma(ch):
                xch = xch_pool.tile([128, 4 * TC * BL], wdt, tag="xch")
                for dc in range(4):
                    nc.gpsimd.dma_start(
                        xch[:, dc * TC * BL:(dc + 1) * TC * BL],
                        xT_d[dc, :, ch * TC:(ch + 1) * TC, :])
                return xch

            def xg_off(jc):
                g_idx, hc = jc // 4, jc % 4
                return (hc // 2) * 128 + BLK2[g_idx] * 32 + (hc % 2) * 16

            def proj_bias_mm(pp, jc):
                # rank-1: gates_j += bias_j (x) ones  (k = 1 partition)
                nc.tensor.matmul(
                    pp[:], brow_sb[:, jc * 128:(jc + 1) * 128], ones_sb[:],
                    start=True, stop=False, skip_group_check=True)

            def proj_mm(pp, xch, jc, dc):
                nc.tensor.matmul(
                    pp[:],
                    wih_sb[:, dc * G + jc * 128: dc * G + (jc + 1) * 128],
                    xch[:, dc * TC * BL:(dc + 1) * TC * BL],
                    start=False, stop=(dc == 3), skip_group_check=True)

            def proj_evac(pp, xg, jc):
                off = xg_off(jc)
                dst = xg[:].rearrange("p (t c) -> p t c", c=256)[:, :, off:off + 16]
                nc.vector.tensor_copy(dst, pp[:])

            def linear_group(ring_src, ch_src, jc):
                lp = lps_pool.tile([128, TC * BL], F32, tag="lp")
                r3 = ring_src[:].rearrange("p (t c) -> p t c", c=64)
                for kc in range(4):
                    nc.tensor.matmul(
                        lp[:],
                        w21_sb[:, kc * H + jc * 128: kc * H + (jc + 1) * 128],
                        r3[:, :, kc * 16:(kc + 1) * 16],
                        start=(kc == 0), stop=(kc == 3))
                linear_evac(lp, ch_src, jc)

            def linear_evac(lp, ch_src, jc):
                ev = evac_pool.tile([128, TC * BL], F32, tag="ev")
                nc.scalar.activation(ev[:], lp[:], AFT.Identity,
                                     bias=b21_sb[:, jc:jc + 1])
                nc.gpsimd.dma_start(
                    pred_d[jc * 128:(jc + 1) * 128,
                           ch_src * TC * BL:(ch_src + 1) * TC * BL], ev[:])

            # per-half elementwise: gates psum -> h/2 half in ring (bf16).
            # Cheap-dispatch ops only; cm runs on GpSimd in parallel with
            # t1h on Vector.  h is stored as h/2 (host pre-scales Whh/W21
            # by 2 and halves h0), so every mul fits the native
            # (in0 - 0.5) * in1 tensor-scalar-tensor form.
            def elem_half(gps, c_prev, c_new, ring, tl, hf):
                acts = acts_pool.tile([128, 128], F32, tag="acts")
                nc.scalar.activation(acts[:], gps[:, 0:128], AFT.Sigmoid)
                i_, f_ = acts[:, 0:32], acts[:, 32:64]
                o_, g_ = acts[:, 64:96], acts[:, 96:128]
                cs = slice(hf * 32, (hf + 1) * 32)
                t1h = small_pool.tile([128, 32], F32, tag="t1h_%d" % hf)
                # t1h = (sig(2g) - 0.5) * sig(i)  (= tanh(g)*sig(i)/2)
                nc.vector.scalar_tensor_tensor(t1h[:], g_, 0.5, i_,
                                               ALU.subtract, ALU.mult)
                cm = small_pool.tile([128, 32], F32, tag="cm_%d" % hf)
                nc.gpsimd.tensor_tensor(cm[:], f_, c_prev[:, cs], ALU.mult)
                # c_new = 2*t1h + cm
                nc.vector.affine_then_add(c_new[:, cs], t1h[:], cm[:], 2.0, 0.0)
                s2c = small_pool.tile([128, 32], F32, tag="s2c_%d" % hf)
                nc.scalar.activation(s2c[:], c_new[:, cs], AFT.Sigmoid, scale=2.0)
                # h/2 = (s2c - 0.5) * sig(o) -> bf16
                nc.vector.scalar_tensor_tensor(
                    ring[:, tl * 64 + hf * 32: tl * 64 + (hf + 1) * 32],
                    s2c[:], 0.5, o_, ALU.subtract, ALU.mult)

            # j-groups per half: half 0 -> hc 0,1 ; half 1 -> hc 2,3
            HALF_JC = {0: [g * 4 + hc for g in range(4) for hc in (0, 1)],
                       1: [g * 4 + hc for g in range(4) for hc in (2, 3)]}

            def run_once():
                # ---- prologue: project chunk 0 ----
                xch = proj_dma(0)
                xg_cur = xg_pool.tile([128, TC * 256], wdt, tag="xg")
                for jc in range(16):
                    pp = pps_pool.tile([128, TC * BL], F32, tag="pp")
                    proj_bias_mm(pp, jc)
                    for dc in range(4):
                        proj_mm(pp, xch, jc, dc)
                    proj_evac(pp, xg_cur, jc)

                c_prev = c0_sb
                prev_ring = None
                xg_next = None
                pp_cur = None
                for ch in range(chunks):
                    ring = ring_pool.tile([128, TC * 64], wdt, tag="ring")
                    for tl in range(TC):
                        if tl > 0:
                            hsrc, hoff = ring, (tl - 1) * 64
                        elif ch > 0:
                            hsrc, hoff = prev_ring, (TC - 1) * 64
                        else:
                            hsrc, hoff = h0_sb, 0
                        have_proj = ch + 1 < chunks
                        if have_proj and tl == 0:
                            xch = proj_dma(ch + 1)
                            xg_next = xg_pool.tile([128, TC * 256], wdt, tag="xg")

                        # ---- psum preload: xg_t via identity matmul ----
                        # full-bank tiles: a start=True matmul clears its
                        # whole PSUM bank, so two rotation bufs sharing one
                        # bank would serialize preload(t+1) behind the
                        # sigmoid reads of step t.  Pad to 2KB/partition.
                        gpsA = gpsA_pool.tile([128, 512], F32, tag="ga")
                        gpsB = gpsB_pool.tile([128, 512], F32, tag="gb")
                        xbase = tl * 256
                        nc.tensor.matmul(gpsA[:, 0:128], iden_sb[:],
                                         xg_cur[:, xbase:xbase + 128],
                                         start=True, stop=False)
                        nc.tensor.matmul(gpsB[:, 0:128], iden_sb[:],
                                         xg_cur[:, xbase + 128:xbase + 256],
                                         start=True, stop=False)

                        # ---- proj filler piece 1 (next chunk) ----
                        if have_proj:
                            pidx = tl * 2
                            jc_f, dc_f = pidx // 4, pidx % 4
                            if dc_f == 0:
                                pp_cur = pps_pool.tile([128, TC * BL], F32, tag="pp")
                                proj_bias_mm(pp_cur, jc_f)
                            proj_mm(pp_cur, xch, jc_f, dc_f)

                        # ---- recurrence matmuls, (k01 | k23) x j-half ----
                        for hf in (0, 1):
                            gps = gpsA if hf == 0 else gpsB
                            jcs = HALF_JC[hf]
                            for kpair in ((0, 1), (2, 3)):
                                for kc in kpair:
                                    for j_i, jc in enumerate(jcs):
                                        g_idx, hc = jc // 4, jc % 4
                                        off = BLK2[g_idx] * 32 + (hc % 2) * 16
                                        last = (kc == 3 and j_i == len(jcs) - 1)
                                        nc.tensor.matmul(
                                            gps[:, off:off + 16],
                                            whh_sb[:, kc * G + jc * 128:
                                                   kc * G + (jc + 1) * 128],
                                            hsrc[:, hoff + kc * 16:
                                                 hoff + (kc + 1) * 16],
                                            start=False, stop=last,
                                            skip_group_check=True)

                        # ---- proj filler piece 2 ----
                        if have_proj:
                            pidx = tl * 2 + 1
                            jc_f, dc_f = pidx // 4, pidx % 4
                            proj_mm(pp_cur, xch, jc_f, dc_f)
                            if dc_f == 3:
                                proj_evac(pp_cur, xg_next, jc_f)

                        # ---- linear filler (previous chunk), 1 mm/step ----
                        if ch >= 1 and 3 <= tl < 19:
                            li = tl - 3
                            jc_l, kc_l = li // 4, li % 4
                            if kc_l == 0:
                                lp_cur = lps_pool.tile([128, TC * BL], F32, tag="lp")
                            r3 = prev_ring[:].rearrange("p (t c) -> p t c", c=64)
                            nc.tensor.matmul(
                                lp_cur[:],
                                w21_sb[:, kc_l * H + jc_l * 128:
                                       kc_l * H + (jc_l + 1) * 128],
                                r3[:, :, kc_l * 16:(kc_l + 1) * 16],
                                start=(kc_l == 0), stop=(kc_l == 3))
                            if kc_l == 3:
                                linear_evac(lp_cur, ch - 1, jc_l)

                        # ---- gate nonlinearities + state update (halves) ----
                        c_new = c_pool.tile([128, 64], F32, tag="c")
                        elem_half(gpsA, c_prev, c_new, ring, tl, 0)
                        elem_half(gpsB, c_prev, c_new, ring, tl, 1)
                        c_prev = c_new
                    prev_ring = ring
                    if ch + 1 < chunks:
                        xg_cur = xg_next
                # epilogue: linear for the last chunk
                for jc in range(4):
                    linear_group(prev_ring, chunks - 1, jc)


            for _rep in range(repeats):
                run_once()

    nc.compile()
    return nc


@functools.lru_cache(maxsize=4)
def _get_program(chunks=None, repeats=1):
    return _build_program(chunks, repeats)


def _pack_core_inputs(x, h0, c0, Wih, Whh, bias, W21, b21_or_zero):
    """Host-side layout prep for one core. x:[BL,T,D], h0/c0:[BL,H]."""
    npw = _np_wdt()
    # g-gate rows [2H,3H) pre-scaled by 2: tanh(g) computed as 2*sig(2g)-1
    Wih = np.concatenate([Wih[:2 * H], Wih[2 * H:3 * H] * 2.0, Wih[3 * H:]])
    Whh = np.concatenate([Whh[:2 * H], Whh[2 * H:3 * H] * 2.0, Whh[3 * H:]])
    bias = np.concatenate([bias[:2 * H], bias[2 * H:3 * H] * 2.0, bias[3 * H:]])
    # h is stored as h/2 in-kernel: scale the h-consuming weights by 2
    Whh = Whh * 2.0
    W21 = W21 * 2.0
    h0 = h0 * 0.5
    xT = np.ascontiguousarray(
        x.transpose(2, 1, 0).reshape(4, 128, T, BL)).astype(npw)
    wih = np.ascontiguousarray(Wih.T.reshape(4, 128, G)).astype(npw)
    whh = np.ascontiguousarray(Whh.T.reshape(4, 128, G)).astype(ml_dtypes.float8_e4m3fn)
    w21 = np.ascontiguousarray(W21.T.reshape(4, 128, H)).astype(npw)
    biasg = np.ascontiguousarray(bias.reshape(16, 128).T).astype(np.float32)
    brow = np.ascontiguousarray(bias.reshape(1, G)).astype(npw)
    ones = np.ones((1, TC * BL), npw)
    b21v = np.ascontiguousarray(b21_or_zero.reshape(4, 128).T).astype(np.float32)
    h0p = np.ascontiguousarray(
        h0.T.reshape(4, 128, BL).transpose(1, 0, 2).reshape(128, 64)).astype(npw)
    c0p = np.ascontiguousarray(
        c0.T.reshape(4, 128, BL).transpose(1, 0, 2).reshape(128, 64)).astype(np.float32)
    iden = np.eye(128, dtype=npw)
    return {"xT": xT, "wih": wih, "whh": whh, "w21": w21, "biasg": biasg,
            "brow": brow, "ones": ones,
            "b21": b21v, "h0p": h0p, "c0p": c0p, "iden": iden}


def _make_in_maps(inputs):
    f32 = np.float32
    x_f = np.asarray(inputs["x_f"], f32)
    x_b = np.asarray(inputs["x_b"], f32)
    h0_f, c0_f = np.asarray(inputs["h0_f"], f32), np.asarray(inputs["c0_f"], f32)
    h0_b, c0_b = np.asarray(inputs["h0_b"], f32), np.asarray(inputs["c0_b"], f32)
    Wih_f, Whh_f = np.asarray(inputs["Wih_f"], f32), np.asarray(inputs["Whh_f"], f32)
    Wih_b, Whh_b = np.asarray(inputs["Wih_b"], f32), np.asarray(inputs["Whh_b"], f32)
    bias_f = np.asarray(inputs["bih_f"], f32) + np.asarray(inputs["bhh_f"], f32)
    bias_b = np.asarray(inputs["bih_b"], f32) + np.asarray(inputs["bhh_b"], f32)
    W1, b1 = np.asarray(inputs["W1"], f32), np.asarray(inputs["b1"], f32)
    W2, b2 = np.asarray(inputs["W2"], f32), np.asarray(inputs["b2"], f32)

    W21 = (W2 @ W1).astype(f32)          # pred = out @ W21.T + b21
    b21 = (W2 @ b1 + b2).astype(f32)
    zeros = np.zeros_like(b21)

    in_maps = []
    for r in range(N_CORES):
        d, s = r % 2, r // 2
        sl = slice(s * BL, (s + 1) * BL)
        if d == 0:
            in_maps.append(_pack_core_inputs(
                x_f[sl], h0_f[sl], c0_f[sl], Wih_f, Whh_f, bias_f, W21, b21))
        else:
            in_maps.append(_pack_core_inputs(
                x_b[sl], h0_b[sl], c0_b[sl], Wih_b, Whh_b, bias_b, W21, zeros))
    return in_maps


def _assemble(results):
    out = np.empty((B, T, H), np.float32)
    for s in range(N_CORES // 2):
        sT = results[2 * s]["predT"] + results[2 * s + 1]["predT"]
        out[s * BL:(s + 1) * BL] = sT.reshape(H, T, BL).transpose(2, 1, 0)
    return out.reshape(B * T, H)


def kernel(x_f, x_b, h0_f, c0_f, h0_b, c0_b,
           Wih_f, Whh_f, bih_f, bhh_f,
           Wih_b, Whh_b, bih_b, bhh_b,
           W1, b1, W2, b2):
    in_maps = _make_in_maps(dict(
        x_f=x_f, x_b=x_b, h0_f=h0_f, c0_f=c0_f, h0_b=h0_b, c0_b=c0_b,
        Wih_f=Wih_f, Whh_f=Whh_f, bih_f=bih_f, bhh_f=bhh_f,
        Wih_b=Wih_b, Whh_b=Whh_b, bih_b=bih_b, bhh_b=bhh_b,
        W1=W1, b1=b1, W2=W2, b2=b2))
    nc = _get_program()
    res = run_bass_kernel_spmd(nc, in_maps, core_ids=list(range(N_CORES)))
    return _assemble(res.results)

